# revision 16
# baseline (speedup 1.0000x reference)
"""Bass/Tile Trainium2 kernel for nn_CrossAttention (B=4, Nq=Nk=2048, D=1024, H=16).

Sharding v3 (batch x head-group): core c handles batch b=c//2 and head-group
g=c%2 (8 of 16 heads), over ALL 2048 query rows of its batch. Each core
computes a partial output  out_c = concat_h(att_h) @ Wp.T[g*512:(g+1)*512, :]
(+ bp on g==0 cores); the host sums the two partials per batch. This removes
the duplicated K/V projections the batch x query-split sharding had.

Ragged-sequence optimization: valid keys (attention_mask==1) are packed on the
host, so the kernel only attends over ~Nk/2 keys; pad rows get a -60 additive
bias before exp. Only the last `nbias` key tiles can contain pad.

Schedule (all matmul operands bf16, fp32 PSUM accumulation):
  startup DMAs split fine-grained and issued across 4 engine sequencers so the
  first V-proj matmul starts early.
  V proj upfront; K projections for head-pair hp+1 interleaved between the
  j=0 attention segments so the scalar engine's EXP stream starts ~40us in;
  Q projection is chunk-outer (all 4 head-pairs per 512-q chunk) emitted just
  before that q-chunk's segments, with xq streamed through a 2-buffer ring.
  Attention q-chunk outer, head-pair inner; AV of the previous segment is
  issued interleaved between this segment's score kp-groups (keeps the PE fed
  while EXP catches up); out-proj of q-chunk j streamed after its last AV.
"""
import numpy as np
import ml_dtypes

import concourse.bass as bass
import concourse.mybir as mybir
import concourse.tile as tile
from concourse import bacc
from concourse.bass_utils import run_bass_kernel_spmd

F32 = mybir.dt.float32
BF16 = mybir.dt.bfloat16
NPBF16 = ml_dtypes.bfloat16

B, NQ_FULL, NK_FULL, D, H, DH = 4, 2048, 2048, 1024, 16, 64
SCALE = DH ** -0.5
MASK_NEG = -60.0  # additive bias (post-scale) for pad keys; exp(-60) ~ 9e-27
N_CORES = 8
HG = H // 2        # heads per core


def _chunks(n, w=512):
    out, j = [], 0
    while j < n:
        out.append((j, min(w, n - j)))
        j += min(w, n - j)
    return out


def build_nc(nq, nk, nbias=2, d=D):
    dh = DH
    assert dh == 64 and d % 128 == 0 and nk % 128 == 0 and nq % 512 == 0
    IC = d // 128          # contraction chunks for QKV projections
    NKT = nk // 128        # key tiles
    QC = nq // 512         # q chunks
    QW = 512
    HP = HG // 2           # head pairs per core
    DC = HG * dh // 128    # contraction chunks for out-proj (4)
    nbias = min(nbias, NKT)
    kchunks = _chunks(nk)
    ochunks = _chunks(d)

    nc = bacc.Bacc("TRN2", target_bir_lowering=False, debug=False)

    xq = nc.declare_dram_parameter("xq", [d, nq], BF16, isOutput=False)
    xk = nc.declare_dram_parameter("xk", [d, nk], BF16, isOutput=False)
    # xv arrives kt-major from the host: [kt*128+p, ic*128+n] = v_T[ic*128+p, kt*128+n]
    xv = nc.declare_dram_parameter("xv", [nk, d], BF16, isOutput=False)
    wq = nc.declare_dram_parameter("wq", [d, HG * dh], BF16, isOutput=False)
    wk = nc.declare_dram_parameter("wk", [d, HG * dh], BF16, isOutput=False)
    wv = nc.declare_dram_parameter("wv", [d, HG * dh], BF16, isOutput=False)
    wp = nc.declare_dram_parameter("wp", [HG * dh, d], BF16, isOutput=False)
    maskb = nc.declare_dram_parameter("maskb", [128, NKT], F32, isOutput=False)
    bpb = nc.declare_dram_parameter("bpb", [1, d], F32, isOutput=False)
    out = nc.declare_dram_parameter("out", [nq, d], F32, isOutput=True)

    with tile.TileContext(nc) as tc:
        with (
            tc.tile_pool(name="const", bufs=1) as cpool,
            tc.tile_pool(name="acts", bufs=1) as apool,
            tc.tile_pool(name="wpool", bufs=1) as wpool,
            tc.tile_pool(name="xk_p", bufs=1) as xkpool,
            tc.tile_pool(name="xq_p", bufs=2) as xqpool,
        ):
            avT_s = apool.tile([128, DC, nq], BF16, tag="avT")
            vh_s = apool.tile([128, NKT, HG, 128], BF16, tag="vh")
            kh_all = apool.tile([128, HP, nk], BF16, tag="kh")
            qh_all = apool.tile([128, HP, nq], BF16, tag="qh")

            # round-robin DMA issuance over the DMA-capable engine sequencers
            issuers = [nc.sync, nc.scalar, nc.gpsimd]
            istate = [0]

            def dma(out_ap, in_ap):
                eng = issuers[istate[0] % len(issuers)]
                istate[0] += 1
                eng.dma_start(out=out_ap, in_=in_ap)

            def load_w(wdram, wid, ncols):
                w_s = wpool.tile([128, IC, ncols], BF16, tag=wid, name=wid)
                for ic in range(IC):
                    dma(w_s[:, ic, :], wdram[ic * 128:(ic + 1) * 128, :])
                return w_s

            def load_xT(pool, xdram, n, name):
                xT = pool.tile([128, IC, n], BF16, tag=name, name=name)
                h = n // 2
                for ic in range(IC):
                    dma(xT[:, ic, 0:h], xdram[ic * 128:(ic + 1) * 128, 0:h])
                    dma(xT[:, ic, h:n], xdram[ic * 128:(ic + 1) * 128, h:n])
                return xT

            xq_tiles = {}

            def load_xq_chunk(j):
                xT = xqpool.tile([128, IC, QW], BF16, tag="xq", name=f"xq{j}")
                for ic in range(IC):
                    dma(xT[:, ic, :], xq[ic * 128:(ic + 1) * 128, j * QW:(j + 1) * QW])
                xq_tiles[j] = xT

            xv_cm = tc.tile_pool(name="xv_p", bufs=1)
            xvpool = xv_cm.__enter__()

            # DMA issue order = consumption order: V-proj inputs first.
            # xv is loaded per key-tile (all IC slabs per kt in one strided DMA
            # pair) so the first V-proj matmul only waits for its own key tile.
            wv_s = load_w(wv, "wv_s", HG * dh)
            xvd = xv.rearrange("(a p) n -> p a n", p=128)
            xv_tiles = []
            for kt in range(NKT):
                k0 = kt * 128
                xkt = xvpool.tile([128, IC, 128], BF16, tag=f"xv{kt}",
                                  name=f"xv{kt}")
                dma(xkt[:, :, 0:64], xvd[:, :, k0:k0 + 64])
                dma(xkt[:, :, 64:128], xvd[:, :, k0 + 64:k0 + 128])
                xv_tiles.append(xkt)
            wk_s = load_w(wk, "wk_s", HG * dh)
            xkT = load_xT(xkpool, xk, nk, "xkT")
            wq_s = load_w(wq, "wq_s", HG * dh)
            load_xq_chunk(0)
            load_xq_chunk(1)
            maskb_s = cpool.tile([128, NKT], F32, tag="maskb")
            dma(maskb_s[:, :], maskb[:, :])
            bp_row = cpool.tile([1, d], F32, tag="bp_row")
            dma(bp_row[:, :], bpb[:, :])
            bp_s = cpool.tile([128, d], F32, tag="bp")
            nc.gpsimd.partition_broadcast(bp_s[:, :], bp_row[:, :])
            wp_s = wpool.tile([128, DC, d], BF16, tag="wp_s", name="wp_s")
            for dc in range(DC):
                dma(wp_s[:, dc, :], wp[dc * 128:(dc + 1) * 128, :])

            nc.gpsimd.memset(vh_s[:, :, :, dh:], 1.0)
            scratch = cpool.tile([128, 512], BF16, tag="scratch")
            nc.gpsimd.memset(scratch[:, :], 0.0)

            with tc.tile_pool(name="mm_ps", bufs=2, space="PSUM") as mmps:
                # HAM warm-up: junk matmuls keep the PE busy while the first
                # input DMAs land, so real matmuls start at the warm clock.
                wps = mmps.tile([128, 512], F32, tag="ps", name="warm")
                for w in range(14):
                    nc.tensor.matmul(
                        wps[:, :], scratch[:, 0:128], scratch[:, :],
                        start=True, stop=True,
                    )
                # ---- V projection (8 heads, N=512) ----
                for kt in range(NKT):
                    ps = mmps.tile([128, 512], F32, tag="ps", name=f"vps{kt}")
                    for ic in range(IC):
                        nc.tensor.matmul(
                            ps[:, :],
                            xv_tiles[kt][:, ic, :],
                            wv_s[:, ic, :],
                            start=(ic == 0), stop=(ic == IC - 1),
                        )
                    nc.vector.tensor_copy(
                        vh_s[:, kt, :, 0:dh],
                        ps[:, :].rearrange("p (a b) -> p a b", b=dh),
                    )
                xv_cm.__exit__(None, None, None)

                with (
                    tc.tile_pool(name="epool", bufs=4) as epool,
                    tc.tile_pool(name="sc_ps", bufs=2, space="PSUM") as scps,
                    tc.tile_pool(name="av_ps", bufs=2, space="PSUM") as avps,
                    tc.tile_pool(name="rpool", bufs=3) as rpool,
                    tc.tile_pool(name="avn", bufs=2) as avnpool,
                    tc.tile_pool(name="obuf", bufs=3) as obuf,
                ):
                    def k_proj(hp):
                        for j0, jw in kchunks:
                            ps = mmps.tile([128, 512], F32, tag="ps",
                                           name=f"kps{hp}_{j0}")
                            for ic in range(IC):
                                nc.tensor.matmul(
                                    ps[:, :jw],
                                    wk_s[:, ic, hp * 128:(hp + 1) * 128],
                                    xkT[:, ic, j0:j0 + jw],
                                    start=(ic == 0), stop=(ic == IC - 1),
                                )
                            nc.vector.tensor_copy(kh_all[:, hp, j0:j0 + jw],
                                                  ps[:, :jw])

                    def q_proj_chunk(j):
                        xT = xq_tiles[j]
                        for hp in range(HP):
                            ps = mmps.tile([128, 512], F32, tag="ps",
                                           name=f"qps{hp}_{j}")
                            for ic in range(IC):
                                nc.tensor.matmul(
                                    ps[:, :],
                                    wq_s[:, ic, hp * 128:(hp + 1) * 128],
                                    xT[:, ic, :],
                                    start=(ic == 0), stop=(ic == IC - 1),
                                )
                            nc.vector.tensor_copy(
                                qh_all[:, hp, j * QW:(j + 1) * QW], ps[:, :])

                    def emit_scores_kp(j, hp, es, kp):
                        q0 = j * QW
                        kts = [kt for kt in (2 * kp, 2 * kp + 1) if kt < NKT]
                        pss = [
                            scps.tile([128, 2, 512], F32, tag="sc",
                                      name=f"sc{hp}_{j}_{kp}_{hf}")
                            for hf in range(2)
                        ]
                        for si, kt in enumerate(kts):
                            for half, ps in enumerate(pss):
                                p0 = half * 64
                                nc.tensor.matmul(
                                    ps[:, si, :QW],
                                    kh_all[p0:p0 + 64, hp, kt * 128:(kt + 1) * 128],
                                    qh_all[p0:p0 + 64, hp, q0:q0 + QW],
                                    start=True, stop=True,
                                    tile_position=(p0, 0),
                                )
                        for half, e in enumerate(es):
                            ps = pss[half]
                            if kts[-1] < NKT - nbias and len(kts) == 2:
                                nc.scalar.activation(
                                    e[:, kts[0]:kts[0] + 2, :], ps[:, :, :QW],
                                    mybir.ActivationFunctionType.Exp,
                                    bias=0.0, scale=SCALE,
                                )
                            else:
                                for si, kt in enumerate(kts):
                                    if kt >= NKT - nbias:
                                        nc.scalar.activation(
                                            e[:, kt, :], ps[:, si, :QW],
                                            mybir.ActivationFunctionType.Exp,
                                            bias=maskb_s[:, kt:kt + 1], scale=SCALE,
                                        )
                                    else:
                                        nc.scalar.activation(
                                            e[:, kt, :], ps[:, si, :QW],
                                            mybir.ActivationFunctionType.Exp,
                                            bias=0.0, scale=SCALE,
                                        )

                    def av_half(j, hp, es, half):
                        q0 = j * QW
                        e = es[half]
                        hh = 2 * hp + half
                        av = avps.tile([128, 512], F32, tag="av",
                                       name=f"av{hp}_{j}_{half}")
                        for kt in range(NKT):
                            nc.tensor.matmul(
                                av[:, :QW],
                                vh_s[:, kt, hh, :],
                                e[:, kt, :],
                                start=(kt == 0), stop=(kt == NKT - 1),
                            )
                        d64 = rpool.tile([128, QW], F32, tag="d64",
                                         name=f"d64_{hp}_{j}_{half}")
                        nc.vector.tensor_copy(d64[64:128, :], av[64:128, :QW])
                        d0 = rpool.tile([64, QW], F32, tag="d0",
                                        name=f"d0_{hp}_{j}_{half}")
                        nc.gpsimd.dma_start(out=d0[:, :], in_=d64[64:128, :])
                        rbb = rpool.tile([dh, QW], F32, tag="rbb",
                                         name=f"rbb_{hp}_{j}_{half}")
                        nc.vector.reciprocal_approx_fast(out=rbb[:, :], in_=d0[:, :])
                        if half == 0:
                            nc.vector.tensor_mul(
                                avT_s[0:dh, hp, q0:q0 + QW], av[0:dh, :QW], rbb[:, :]
                            )
                        else:
                            avn = avnpool.tile([dh, QW], BF16, tag="avn",
                                               name=f"avn{hp}_{j}")
                            nc.vector.tensor_mul(avn[:, :], av[0:dh, :QW], rbb[:, :])
                            nc.gpsimd.dma_start(
                                out=avT_s[64:128, hp, q0:q0 + 256], in_=avn[:, 0:256]
                            )
                            nc.gpsimd.dma_start(
                                out=avT_s[64:128, hp, q0 + 256:q0 + QW],
                                in_=avn[:, 256:QW]
                            )

                    def out_proj(j):
                        nsplit = 4 if j == QC - 1 else 1
                        for qt in range(QW // 128):
                            qa = j * QW + qt * 128
                            pso = []
                            for oc, (o0, ow) in enumerate(ochunks):
                                pso.append(mmps.tile([128, 512], F32, tag="ps",
                                                     name=f"o{j}_{qt}_{o0}"))
                            for dc in range(DC):
                                for oc, (o0, ow) in enumerate(ochunks):
                                    nc.tensor.matmul(
                                        pso[oc][:, :ow],
                                        avT_s[:, dc, qa:qa + 128],
                                        wp_s[:, dc, o0:o0 + ow],
                                        start=(dc == 0), stop=(dc == DC - 1),
                                    )
                            for oc, (o0, ow) in enumerate(ochunks):
                                ot = obuf.tile([128, 512], F32, tag="ot",
                                               name=f"ot{j}_{qt}_{o0}")
                                nc.vector.tensor_add(
                                    ot[:, :ow], pso[oc][:, :ow], bp_s[:, o0:o0 + ow]
                                )
                                sw = ow // nsplit
                                for si in range(nsplit):
                                    nc.gpsimd.dma_start(
                                        out=out[qa:qa + 128,
                                                o0 + si * sw:o0 + (si + 1) * sw],
                                        in_=ot[:, si * sw:(si + 1) * sw]
                                    )

                    NKP = (NKT + 1) // 2
                    outq = []

                    def emit_segment(j, hp, prev):
                        if hp == 2 and outq:
                            out_proj(outq.pop(0))
                        es = [
                            epool.tile([128, NKT, QW], BF16, tag="e",
                                       name=f"e{hp}_{j}_{hf}")
                            for hf in range(2)
                        ]
                        fill = {min(1, NKP - 1): 0, min(3, NKP - 1): 1}
                        for kp in range(NKP):
                            emit_scores_kp(j, hp, es, kp)
                            if prev is not None and kp in fill:
                                av_half(prev[0], prev[1], prev[2], fill[kp])
                        if prev is not None and prev[1] == HP - 1:
                            outq.append(prev[0])
                        return es

                    k_proj(0)
                    q_proj_chunk(0)
                    prev = None
                    for j in range(QC):
                        for hp in range(HP):
                            es = emit_segment(j, hp, prev)
                            prev = (j, hp, es)
                            if j == 0 and hp + 1 < HP:
                                k_proj(hp + 1)
                        if j + 1 < QC:
                            q_proj_chunk(j + 1)
                            if j + 2 < QC:
                                load_xq_chunk(j + 2)
                    av_half(*prev, 0)
                    av_half(*prev, 1)
                    outq.append(prev[0])
                    for j in outq:
                        out_proj(j)

    nc.compile()
    return nc


def host_prep(q, k, v, attention_mask, Wq, Wk, Wv, Wp, bp):
    """Pack valid keys, slice + cast full inputs into per-core input maps.
    Core c: batch c//2, head-group c%2."""
    bsz, nk_full = attention_mask.shape
    idxs = [np.flatnonzero(attention_mask[b]) for b in range(bsz)]
    nv_min = min(len(ix) for ix in idxs)
    nk = max(128, -(-max(len(ix) for ix in idxs) // 128) * 128)
    nkt = nk // 128
    nbias = max(1, -(-(nk - nv_min) // 128))

    wqT = np.ascontiguousarray(Wq.T).astype(NPBF16)
    wkT = np.ascontiguousarray(Wk.T).astype(NPBF16)
    wvT = np.ascontiguousarray(Wv.T).astype(NPBF16)
    wpT = np.ascontiguousarray(Wp.T).astype(NPBF16)
    G = HG * DH

    packed = []
    for b in range(bsz):
        ix = idxs[b]
        kp = np.zeros((nk, k.shape[2]), NPBF16)
        vp = np.zeros((nk, v.shape[2]), NPBF16)
        kp[:len(ix)] = k[b][ix].astype(NPBF16)
        vp[:len(ix)] = v[b][ix].astype(NPBF16)
        mb = np.full(nk, MASK_NEG, np.float32)
        mb[:len(ix)] = 0.0
        packed.append((np.ascontiguousarray(kp.T), np.ascontiguousarray(vp.T),
                       np.ascontiguousarray(mb.reshape(nkt, 128).T),
                       np.ascontiguousarray(q[b].astype(NPBF16).T)))

    bp_zero = np.zeros((1, D), np.float32)
    bp_real = np.ascontiguousarray(bp[None, :]).astype(np.float32)
    in_maps = []
    for c in range(N_CORES):
        b, g = divmod(c, 2)
        kp, vp, mb, xq = packed[b]
        in_maps.append({
            "xq": xq, "xk": kp, "xv": vp,
            "wq": np.ascontiguousarray(wqT[:, g * G:(g + 1) * G]),
            "wk": np.ascontiguousarray(wkT[:, g * G:(g + 1) * G]),
            "wv": np.ascontiguousarray(wvT[:, g * G:(g + 1) * G]),
            "wp": np.ascontiguousarray(wpT[g * G:(g + 1) * G, :]),
            "maskb": mb, "bpb": bp_real if g == 0 else bp_zero,
        })
    return in_maps, nk, nbias


_NC_CACHE = {}


def get_nc(nq, nk, nbias=2):
    key = (nq, nk, nbias)
    if key not in _NC_CACHE:
        _NC_CACHE[key] = build_nc(nq, nk, nbias)
    return _NC_CACHE[key]


def unshard(results):
    out = np.empty((B, NQ_FULL, D), np.float32)
    for b in range(B):
        out[b] = results[2 * b]["out"]
        out[b] += results[2 * b + 1]["out"]
    return out


def kernel(q, k, v, attention_mask, Wq, Wk, Wv, Wp, bp):
    in_maps, nk, nbias = host_prep(q, k, v, attention_mask, Wq, Wk, Wv, Wp, bp)
    nc = get_nc(NQ_FULL, nk, nbias)
    res = run_bass_kernel_spmd(nc, in_maps, core_ids=list(range(N_CORES)))
    return unshard(res.results)


# revision 18
# speedup vs baseline: 1.1963x; 1.1963x over previous
"""Bass/Tile Trainium2 kernel for nn_CrossAttention (B=4, Nq=Nk=2048, D=1024, H=16).

Sharding v3 (batch x head-group): core c handles batch b=c//2 and head-group
g=c%2 (8 of 16 heads), over ALL 2048 query rows of its batch. Each core
computes a partial output  out_c = concat_h(att_h) @ Wp.T[g*512:(g+1)*512, :]
(+ bp on g==0 cores); the host sums the two partials per batch. This removes
the duplicated K/V projections the batch x query-split sharding had.

Ragged-sequence optimization: valid keys (attention_mask==1) are packed on the
host, so the kernel only attends over ~Nk/2 keys; pad rows get a -60 additive
bias before exp. Only the last `nbias` key tiles can contain pad.

Schedule (all matmul operands bf16, fp32 PSUM accumulation):
  startup DMAs split fine-grained and issued across 4 engine sequencers so the
  first V-proj matmul starts early.
  V proj upfront; K projections for head-pair hp+1 interleaved between the
  j=0 attention segments so the scalar engine's EXP stream starts ~40us in;
  Q projection is chunk-outer (all 4 head-pairs per 512-q chunk) emitted just
  before that q-chunk's segments, with xq streamed through a 2-buffer ring.
  Attention q-chunk outer, head-pair inner; AV of the previous segment is
  issued interleaved between this segment's score kp-groups (keeps the PE fed
  while EXP catches up); out-proj of q-chunk j streamed after its last AV.
"""
import numpy as np
import ml_dtypes

import concourse.bass as bass
import concourse.mybir as mybir
import concourse.tile as tile
from concourse import bacc
from concourse.bass_utils import run_bass_kernel_spmd

F32 = mybir.dt.float32
BF16 = mybir.dt.bfloat16
NPBF16 = ml_dtypes.bfloat16

B, NQ_FULL, NK_FULL, D, H, DH = 4, 2048, 2048, 1024, 16, 64
SCALE = DH ** -0.5
MASK_NEG = -60.0  # additive bias (post-scale) for pad keys; exp(-60) ~ 9e-27
N_CORES = 8
HG = H // 2        # heads per core


def _chunks(n, w=512):
    out, j = [], 0
    while j < n:
        out.append((j, min(w, n - j)))
        j += min(w, n - j)
    return out


def build_nc(nq, nk, nbias=2, d=D):
    dh = DH
    assert dh == 64 and d % 128 == 0 and nk % 128 == 0 and nq % 512 == 0
    IC = d // 128          # contraction chunks for QKV projections
    NKT = nk // 128        # key tiles
    QC = nq // 512         # q chunks
    QW = 512
    HP = HG // 2           # head pairs per core
    DC = HG * dh // 128    # contraction chunks for out-proj (4)
    nbias = min(nbias, NKT)
    kchunks = _chunks(nk)
    ochunks = _chunks(d)

    nc = bacc.Bacc("TRN2", target_bir_lowering=False, debug=False)

    xq = nc.declare_dram_parameter("xq", [d, nq], BF16, isOutput=False)
    xk = nc.declare_dram_parameter("xk", [d, nk], BF16, isOutput=False)
    # xv arrives kt-major from the host: [kt*128+p, ic*128+n] = v_T[ic*128+p, kt*128+n]
    xv = nc.declare_dram_parameter("xv", [nk, d], BF16, isOutput=False)
    wq = nc.declare_dram_parameter("wq", [d, HG * dh], BF16, isOutput=False)
    wk = nc.declare_dram_parameter("wk", [d, HG * dh], BF16, isOutput=False)
    wv = nc.declare_dram_parameter("wv", [d, HG * dh], BF16, isOutput=False)
    wp = nc.declare_dram_parameter("wp", [HG * dh, d], BF16, isOutput=False)
    maskb = nc.declare_dram_parameter("maskb", [128, NKT], F32, isOutput=False)
    bpb = nc.declare_dram_parameter("bpb", [1, d], F32, isOutput=False)
    out = nc.declare_dram_parameter("out", [nq, d], F32, isOutput=True)

    with tile.TileContext(nc) as tc:
        with (
            tc.tile_pool(name="const", bufs=1) as cpool,
            tc.tile_pool(name="acts", bufs=1) as apool,
            tc.tile_pool(name="wpool", bufs=1) as wpool,
            tc.tile_pool(name="xk_p", bufs=1) as xkpool,
            tc.tile_pool(name="xq_p", bufs=2) as xqpool,
        ):
            avT_s = apool.tile([128, DC, nq], BF16, tag="avT")
            vh_s = apool.tile([128, NKT, HG, 128], BF16, tag="vh")
            kh_all = apool.tile([128, HP, nk], BF16, tag="kh")
            qh_all = apool.tile([128, HP, nq], BF16, tag="qh")

            # round-robin DMA issuance over the DMA-capable engine sequencers
            issuers = [nc.sync, nc.scalar, nc.gpsimd]
            istate = [0]

            def dma(out_ap, in_ap):
                eng = issuers[istate[0] % len(issuers)]
                istate[0] += 1
                eng.dma_start(out=out_ap, in_=in_ap)

            def load_w(wdram, wid, ncols):
                w_s = wpool.tile([128, IC, ncols], BF16, tag=wid, name=wid)
                for ic in range(IC):
                    dma(w_s[:, ic, :], wdram[ic * 128:(ic + 1) * 128, :])
                return w_s

            def load_xT(pool, xdram, n, name):
                xT = pool.tile([128, IC, n], BF16, tag=name, name=name)
                h = n // 2
                for ic in range(IC):
                    dma(xT[:, ic, 0:h], xdram[ic * 128:(ic + 1) * 128, 0:h])
                    dma(xT[:, ic, h:n], xdram[ic * 128:(ic + 1) * 128, h:n])
                return xT

            xq_tiles = {}

            def load_xq_chunk(j):
                xT = xqpool.tile([128, IC, QW], BF16, tag="xq", name=f"xq{j}")
                for ic in range(IC):
                    dma(xT[:, ic, :], xq[ic * 128:(ic + 1) * 128, j * QW:(j + 1) * QW])
                xq_tiles[j] = xT

            xv_cm = tc.tile_pool(name="xv_p", bufs=1)
            xvpool = xv_cm.__enter__()

            # DMA issue order = consumption order: V-proj inputs first.
            # xv is loaded per key-tile (all IC slabs per kt in one strided DMA
            # pair) so the first V-proj matmul only waits for its own key tile.
            wv_s = load_w(wv, "wv_s", HG * dh)
            xv_tiles = []
            for kt in range(NKT):
                k0 = kt * 128
                xkt = xvpool.tile([128, IC, 128], BF16, tag=f"xv{kt}",
                                  name=f"xv{kt}")
                dma(xkt[:, 0:IC // 2, :], xv[k0:k0 + 128, 0:d // 2])
                dma(xkt[:, IC // 2:, :], xv[k0:k0 + 128, d // 2:d])
                xv_tiles.append(xkt)
            wk_s = load_w(wk, "wk_s", HG * dh)
            xkT = load_xT(xkpool, xk, nk, "xkT")
            wq_s = load_w(wq, "wq_s", HG * dh)
            load_xq_chunk(0)
            load_xq_chunk(1)
            maskb_s = cpool.tile([128, NKT], F32, tag="maskb")
            dma(maskb_s[:, :], maskb[:, :])
            bp_row = cpool.tile([1, d], F32, tag="bp_row")
            dma(bp_row[:, :], bpb[:, :])
            bp_s = cpool.tile([128, d], F32, tag="bp")
            nc.gpsimd.partition_broadcast(bp_s[:, :], bp_row[:, :])
            wp_s = wpool.tile([128, DC, d], BF16, tag="wp_s", name="wp_s")
            for dc in range(DC):
                dma(wp_s[:, dc, :], wp[dc * 128:(dc + 1) * 128, :])

            nc.gpsimd.memset(vh_s[:, :, :, dh:], 1.0)
            scratch = cpool.tile([128, 512], BF16, tag="scratch")
            nc.gpsimd.memset(scratch[:, :], 0.0)

            with tc.tile_pool(name="mm_ps", bufs=2, space="PSUM") as mmps:
                # HAM warm-up: junk matmuls keep the PE busy while the first
                # input DMAs land, so real matmuls start at the warm clock.
                wps = mmps.tile([128, 512], F32, tag="ps", name="warm")
                for w in range(14):
                    nc.tensor.matmul(
                        wps[:, :], scratch[:, 0:128], scratch[:, :],
                        start=True, stop=True,
                    )
                # ---- V projection (8 heads, N=512) ----
                for kt in range(NKT):
                    ps = mmps.tile([128, 512], F32, tag="ps", name=f"vps{kt}")
                    for ic in range(IC):
                        nc.tensor.matmul(
                            ps[:, :],
                            xv_tiles[kt][:, ic, :],
                            wv_s[:, ic, :],
                            start=(ic == 0), stop=(ic == IC - 1),
                        )
                    nc.vector.tensor_copy(
                        vh_s[:, kt, :, 0:dh],
                        ps[:, :].rearrange("p (a b) -> p a b", b=dh),
                    )
                xv_cm.__exit__(None, None, None)

                with (
                    tc.tile_pool(name="epool", bufs=4) as epool,
                    tc.tile_pool(name="sc_ps", bufs=2, space="PSUM") as scps,
                    tc.tile_pool(name="av_ps", bufs=2, space="PSUM") as avps,
                    tc.tile_pool(name="rpool", bufs=3) as rpool,
                    tc.tile_pool(name="avn", bufs=2) as avnpool,
                    tc.tile_pool(name="obuf", bufs=3) as obuf,
                ):
                    def k_proj(hp):
                        for j0, jw in kchunks:
                            ps = mmps.tile([128, 512], F32, tag="ps",
                                           name=f"kps{hp}_{j0}")
                            for ic in range(IC):
                                nc.tensor.matmul(
                                    ps[:, :jw],
                                    wk_s[:, ic, hp * 128:(hp + 1) * 128],
                                    xkT[:, ic, j0:j0 + jw],
                                    start=(ic == 0), stop=(ic == IC - 1),
                                )
                            nc.vector.tensor_copy(kh_all[:, hp, j0:j0 + jw],
                                                  ps[:, :jw])

                    def q_proj_chunk(j):
                        xT = xq_tiles[j]
                        for hp in range(HP):
                            ps = mmps.tile([128, 512], F32, tag="ps",
                                           name=f"qps{hp}_{j}")
                            for ic in range(IC):
                                nc.tensor.matmul(
                                    ps[:, :],
                                    wq_s[:, ic, hp * 128:(hp + 1) * 128],
                                    xT[:, ic, :],
                                    start=(ic == 0), stop=(ic == IC - 1),
                                )
                            nc.vector.tensor_copy(
                                qh_all[:, hp, j * QW:(j + 1) * QW], ps[:, :])

                    def emit_scores_kp(j, hp, es, kp):
                        q0 = j * QW
                        kts = [kt for kt in (2 * kp, 2 * kp + 1) if kt < NKT]
                        pss = [
                            scps.tile([128, 2, 512], F32, tag="sc",
                                      name=f"sc{hp}_{j}_{kp}_{hf}")
                            for hf in range(2)
                        ]
                        for si, kt in enumerate(kts):
                            for half, ps in enumerate(pss):
                                p0 = half * 64
                                nc.tensor.matmul(
                                    ps[:, si, :QW],
                                    kh_all[p0:p0 + 64, hp, kt * 128:(kt + 1) * 128],
                                    qh_all[p0:p0 + 64, hp, q0:q0 + QW],
                                    start=True, stop=True,
                                    tile_position=(p0, 0),
                                )
                        for half, e in enumerate(es):
                            ps = pss[half]
                            if kts[-1] < NKT - nbias and len(kts) == 2:
                                nc.scalar.activation(
                                    e[:, kts[0]:kts[0] + 2, :], ps[:, :, :QW],
                                    mybir.ActivationFunctionType.Exp,
                                    bias=0.0, scale=SCALE,
                                )
                            else:
                                for si, kt in enumerate(kts):
                                    if kt >= NKT - nbias:
                                        nc.scalar.activation(
                                            e[:, kt, :], ps[:, si, :QW],
                                            mybir.ActivationFunctionType.Exp,
                                            bias=maskb_s[:, kt:kt + 1], scale=SCALE,
                                        )
                                    else:
                                        nc.scalar.activation(
                                            e[:, kt, :], ps[:, si, :QW],
                                            mybir.ActivationFunctionType.Exp,
                                            bias=0.0, scale=SCALE,
                                        )

                    def av_half(j, hp, es, half):
                        q0 = j * QW
                        e = es[half]
                        hh = 2 * hp + half
                        av = avps.tile([128, 512], F32, tag="av",
                                       name=f"av{hp}_{j}_{half}")
                        for kt in range(NKT):
                            nc.tensor.matmul(
                                av[:, :QW],
                                vh_s[:, kt, hh, :],
                                e[:, kt, :],
                                start=(kt == 0), stop=(kt == NKT - 1),
                            )
                        d64 = rpool.tile([128, QW], F32, tag="d64",
                                         name=f"d64_{hp}_{j}_{half}")
                        nc.vector.tensor_copy(d64[64:128, :], av[64:128, :QW])
                        d0 = rpool.tile([64, QW], F32, tag="d0",
                                        name=f"d0_{hp}_{j}_{half}")
                        nc.gpsimd.dma_start(out=d0[:, :], in_=d64[64:128, :])
                        rbb = rpool.tile([dh, QW], F32, tag="rbb",
                                         name=f"rbb_{hp}_{j}_{half}")
                        nc.vector.reciprocal_approx_fast(out=rbb[:, :], in_=d0[:, :])
                        if half == 0:
                            nc.vector.tensor_mul(
                                avT_s[0:dh, hp, q0:q0 + QW], av[0:dh, :QW], rbb[:, :]
                            )
                        else:
                            avn = avnpool.tile([dh, QW], BF16, tag="avn",
                                               name=f"avn{hp}_{j}")
                            nc.vector.tensor_mul(avn[:, :], av[0:dh, :QW], rbb[:, :])
                            nc.gpsimd.dma_start(
                                out=avT_s[64:128, hp, q0:q0 + 256], in_=avn[:, 0:256]
                            )
                            nc.gpsimd.dma_start(
                                out=avT_s[64:128, hp, q0 + 256:q0 + QW],
                                in_=avn[:, 256:QW]
                            )

                    def out_proj(j):
                        nsplit = 4 if j == QC - 1 else 1
                        for qt in range(QW // 128):
                            qa = j * QW + qt * 128
                            pso = []
                            for oc, (o0, ow) in enumerate(ochunks):
                                pso.append(mmps.tile([128, 512], F32, tag="ps",
                                                     name=f"o{j}_{qt}_{o0}"))
                            for dc in range(DC):
                                for oc, (o0, ow) in enumerate(ochunks):
                                    nc.tensor.matmul(
                                        pso[oc][:, :ow],
                                        avT_s[:, dc, qa:qa + 128],
                                        wp_s[:, dc, o0:o0 + ow],
                                        start=(dc == 0), stop=(dc == DC - 1),
                                    )
                            for oc, (o0, ow) in enumerate(ochunks):
                                ot = obuf.tile([128, 512], F32, tag="ot",
                                               name=f"ot{j}_{qt}_{o0}")
                                nc.vector.tensor_add(
                                    ot[:, :ow], pso[oc][:, :ow], bp_s[:, o0:o0 + ow]
                                )
                                sw = ow // nsplit
                                for si in range(nsplit):
                                    nc.gpsimd.dma_start(
                                        out=out[qa:qa + 128,
                                                o0 + si * sw:o0 + (si + 1) * sw],
                                        in_=ot[:, si * sw:(si + 1) * sw]
                                    )

                    NKP = (NKT + 1) // 2
                    outq = []

                    def emit_segment(j, hp, prev):
                        if hp == 2 and outq:
                            out_proj(outq.pop(0))
                        es = [
                            epool.tile([128, NKT, QW], BF16, tag="e",
                                       name=f"e{hp}_{j}_{hf}")
                            for hf in range(2)
                        ]
                        fill = {min(1, NKP - 1): 0, min(3, NKP - 1): 1}
                        for kp in range(NKP):
                            emit_scores_kp(j, hp, es, kp)
                            if prev is not None and kp in fill:
                                av_half(prev[0], prev[1], prev[2], fill[kp])
                        if prev is not None and prev[1] == HP - 1:
                            outq.append(prev[0])
                        return es

                    k_proj(0)
                    q_proj_chunk(0)
                    prev = None
                    for j in range(QC):
                        for hp in range(HP):
                            es = emit_segment(j, hp, prev)
                            prev = (j, hp, es)
                            if j == 0 and hp + 1 < HP:
                                k_proj(hp + 1)
                        if j + 1 < QC:
                            q_proj_chunk(j + 1)
                            if j + 2 < QC:
                                load_xq_chunk(j + 2)
                    av_half(*prev, 0)
                    av_half(*prev, 1)
                    outq.append(prev[0])
                    for j in outq:
                        out_proj(j)

    nc.compile()
    return nc


def host_prep(q, k, v, attention_mask, Wq, Wk, Wv, Wp, bp):
    """Pack valid keys, slice + cast full inputs into per-core input maps.
    Core c: batch c//2, head-group c%2."""
    bsz, nk_full = attention_mask.shape
    idxs = [np.flatnonzero(attention_mask[b]) for b in range(bsz)]
    nv_min = min(len(ix) for ix in idxs)
    nk = max(128, -(-max(len(ix) for ix in idxs) // 128) * 128)
    nkt = nk // 128
    nbias = max(1, -(-(nk - nv_min) // 128))

    wqT = np.ascontiguousarray(Wq.T).astype(NPBF16)
    wkT = np.ascontiguousarray(Wk.T).astype(NPBF16)
    wvT = np.ascontiguousarray(Wv.T).astype(NPBF16)
    wpT = np.ascontiguousarray(Wp.T).astype(NPBF16)
    G = HG * DH

    IC = D // 128
    packed = []
    for b in range(bsz):
        ix = idxs[b]
        kp = np.zeros((nk, k.shape[2]), NPBF16)
        vp = np.zeros((nk, v.shape[2]), NPBF16)
        kp[:len(ix)] = k[b][ix].astype(NPBF16)
        vp[:len(ix)] = v[b][ix].astype(NPBF16)
        mb = np.full(nk, MASK_NEG, np.float32)
        mb[:len(ix)] = 0.0
        # kt-major xv layout: xvk[kt*128+p, ic*128+n] = v_T[ic*128+p, kt*128+n]
        xvk = np.ascontiguousarray(
            vp.T.reshape(IC, 128, nkt, 128).transpose(2, 1, 0, 3)
            .reshape(nk, D))
        packed.append((np.ascontiguousarray(kp.T), xvk,
                       np.ascontiguousarray(mb.reshape(nkt, 128).T),
                       np.ascontiguousarray(q[b].astype(NPBF16).T)))

    bp_zero = np.zeros((1, D), np.float32)
    bp_real = np.ascontiguousarray(bp[None, :]).astype(np.float32)
    in_maps = []
    for c in range(N_CORES):
        b, g = divmod(c, 2)
        kp, vp, mb, xq = packed[b]
        in_maps.append({
            "xq": xq, "xk": kp, "xv": vp,
            "wq": np.ascontiguousarray(wqT[:, g * G:(g + 1) * G]),
            "wk": np.ascontiguousarray(wkT[:, g * G:(g + 1) * G]),
            "wv": np.ascontiguousarray(wvT[:, g * G:(g + 1) * G]),
            "wp": np.ascontiguousarray(wpT[g * G:(g + 1) * G, :]),
            "maskb": mb, "bpb": bp_real if g == 0 else bp_zero,
        })
    return in_maps, nk, nbias


_NC_CACHE = {}


def get_nc(nq, nk, nbias=2):
    key = (nq, nk, nbias)
    if key not in _NC_CACHE:
        _NC_CACHE[key] = build_nc(nq, nk, nbias)
    return _NC_CACHE[key]


def unshard(results):
    out = np.empty((B, NQ_FULL, D), np.float32)
    for b in range(B):
        out[b] = results[2 * b]["out"]
        out[b] += results[2 * b + 1]["out"]
    return out


def kernel(q, k, v, attention_mask, Wq, Wk, Wv, Wp, bp):
    in_maps, nk, nbias = host_prep(q, k, v, attention_mask, Wq, Wk, Wv, Wp, bp)
    nc = get_nc(NQ_FULL, nk, nbias)
    res = run_bass_kernel_spmd(nc, in_maps, core_ids=list(range(N_CORES)))
    return unshard(res.results)


# revision 21
# speedup vs baseline: 1.2980x; 1.0850x over previous
"""Bass/Tile Trainium2 kernel for nn_CrossAttention (B=4, Nq=Nk=2048, D=1024, H=16).

Sharding v3 (batch x head-group): core c handles batch b=c//2 and head-group
g=c%2 (8 of 16 heads), over ALL 2048 query rows of its batch. Each core
computes a partial output  out_c = concat_h(att_h) @ Wp.T[g*512:(g+1)*512, :]
(+ bp on g==0 cores); the host sums the two partials per batch. This removes
the duplicated K/V projections the batch x query-split sharding had.

Ragged-sequence optimization: valid keys (attention_mask==1) are packed on the
host, so the kernel only attends over ~Nk/2 keys; pad rows get a -60 additive
bias before exp. Only the last `nbias` key tiles can contain pad.

Schedule (all matmul operands bf16, fp32 PSUM accumulation):
  startup DMAs split fine-grained and issued across 4 engine sequencers so the
  first V-proj matmul starts early.
  V proj upfront; K projections for head-pair hp+1 interleaved between the
  j=0 attention segments so the scalar engine's EXP stream starts ~40us in;
  Q projection is chunk-outer (all 4 head-pairs per 512-q chunk) emitted just
  before that q-chunk's segments, with xq streamed through a 2-buffer ring.
  Attention q-chunk outer, head-pair inner; AV of the previous segment is
  issued interleaved between this segment's score kp-groups (keeps the PE fed
  while EXP catches up); out-proj of q-chunk j streamed after its last AV.
"""
import numpy as np
import ml_dtypes

import concourse.bass as bass
import concourse.mybir as mybir
import concourse.tile as tile
from concourse import bacc
from concourse.bass_utils import run_bass_kernel_spmd

F32 = mybir.dt.float32
BF16 = mybir.dt.bfloat16
NPBF16 = ml_dtypes.bfloat16

B, NQ_FULL, NK_FULL, D, H, DH = 4, 2048, 2048, 1024, 16, 64
SCALE = DH ** -0.5
MASK_NEG = -60.0  # additive bias (post-scale) for pad keys; exp(-60) ~ 9e-27
N_CORES = 8
HG = H // 2        # heads per core


def _chunks(n, w=512):
    out, j = [], 0
    while j < n:
        out.append((j, min(w, n - j)))
        j += min(w, n - j)
    return out


def build_nc(nq, nk, nbias=2, d=D):
    dh = DH
    assert dh == 64 and d % 128 == 0 and nk % 128 == 0 and nq % 512 == 0
    IC = d // 128          # contraction chunks for QKV projections
    NKT = nk // 128        # key tiles
    QC = nq // 512         # q chunks
    QW = 512
    HP = HG // 2           # head pairs per core
    DC = HG * dh // 128    # contraction chunks for out-proj (4)
    nbias = min(nbias, NKT)
    kchunks = _chunks(nk)
    ochunks = _chunks(d)

    nc = bacc.Bacc("TRN2", target_bir_lowering=False, debug=False)

    xq = nc.declare_dram_parameter("xq", [d, nq], BF16, isOutput=False)
    xk = nc.declare_dram_parameter("xk", [d, nk], BF16, isOutput=False)
    # xv arrives kt-major from the host: [kt*128+p, ic*128+n] = v_T[ic*128+p, kt*128+n]
    xv = nc.declare_dram_parameter("xv", [nk, d], BF16, isOutput=False)
    wq = nc.declare_dram_parameter("wq", [d, HG * dh], BF16, isOutput=False)
    wk = nc.declare_dram_parameter("wk", [d, HG * dh], BF16, isOutput=False)
    wv = nc.declare_dram_parameter("wv", [d, HG * dh], BF16, isOutput=False)
    wp = nc.declare_dram_parameter("wp", [HG * dh, d], BF16, isOutput=False)
    maskb = nc.declare_dram_parameter("maskb", [128, NKT], F32, isOutput=False)
    bpb = nc.declare_dram_parameter("bpb", [1, d], F32, isOutput=False)
    out = nc.declare_dram_parameter("out", [nq, d], F32, isOutput=True)

    with tile.TileContext(nc) as tc:
        with (
            tc.tile_pool(name="const", bufs=1) as cpool,
            tc.tile_pool(name="acts", bufs=1) as apool,
            tc.tile_pool(name="wpool", bufs=1) as wpool,
            tc.tile_pool(name="xk_p", bufs=1) as xkpool,
            tc.tile_pool(name="xq_p", bufs=2) as xqpool,
        ):
            avT_s = apool.tile([128, DC, nq], BF16, tag="avT")
            vh_s = apool.tile([128, NKT, HG, 128], BF16, tag="vh")
            kh_all = apool.tile([128, HP, nk], BF16, tag="kh")
            qh_all = apool.tile([128, HP, nq], BF16, tag="qh")

            # round-robin DMA issuance over the DMA-capable engine sequencers
            issuers = [nc.sync, nc.scalar, nc.gpsimd]
            istate = [0]

            def dma(out_ap, in_ap):
                eng = issuers[istate[0] % len(issuers)]
                istate[0] += 1
                eng.dma_start(out=out_ap, in_=in_ap)

            def load_w(wdram, wid, ncols):
                w_s = wpool.tile([128, IC, ncols], BF16, tag=wid, name=wid)
                for ic in range(IC):
                    dma(w_s[:, ic, :], wdram[ic * 128:(ic + 1) * 128, :])
                return w_s

            def load_xT(pool, xdram, n, name):
                xT = pool.tile([128, IC, n], BF16, tag=name, name=name)
                h = n // 2
                for ic in range(IC):
                    dma(xT[:, ic, 0:h], xdram[ic * 128:(ic + 1) * 128, 0:h])
                    dma(xT[:, ic, h:n], xdram[ic * 128:(ic + 1) * 128, h:n])
                return xT

            xq_tiles = {}

            def load_xq_chunk(j):
                xT = xqpool.tile([128, IC, QW], BF16, tag="xq", name=f"xq{j}")
                for ic in range(IC):
                    dma(xT[:, ic, :], xq[ic * 128:(ic + 1) * 128, j * QW:(j + 1) * QW])
                xq_tiles[j] = xT

            xv_cm = tc.tile_pool(name="xv_p", bufs=1)
            xvpool = xv_cm.__enter__()

            # tiny consts first (sync queue) so nothing downstream waits long
            maskb_s = cpool.tile([128, NKT], F32, tag="maskb")
            nc.sync.dma_start(out=maskb_s[:, :], in_=maskb[:, :])
            bp_row = cpool.tile([1, d], F32, tag="bp_row")
            nc.sync.dma_start(out=bp_row[:, :], in_=bpb[:, :])
            # scratch memset is gpsimd's FIRST op; the HAM warm-up matmuls
            # depend only on it, so the PE starts ~1us in
            scratch = cpool.tile([128, 512], BF16, tag="scratch")
            nc.gpsimd.memset(scratch[:, :], 0.0)

            # DMA issue order = consumption order: V-proj inputs first.
            # xv is loaded per key-tile (kt-major host layout) so the first
            # V-proj matmul only waits for its own key tile.
            wv_s = load_w(wv, "wv_s", HG * dh)
            xv_tiles = []
            for kt in range(NKT):
                k0 = kt * 128
                xkt = xvpool.tile([128, IC, 128], BF16, tag=f"xv{kt}",
                                  name=f"xv{kt}")
                dma(xkt[:, 0:IC // 2, :], xv[k0:k0 + 128, 0:d // 2])
                dma(xkt[:, IC // 2:, :], xv[k0:k0 + 128, d // 2:d])
                xv_tiles.append(xkt)
            wk_s = load_w(wk, "wk_s", HG * dh)
            xkT = load_xT(xkpool, xk, nk, "xkT")
            wq_s = load_w(wq, "wq_s", HG * dh)
            load_xq_chunk(0)
            load_xq_chunk(1)
            wp_s = wpool.tile([128, DC, d], BF16, tag="wp_s", name="wp_s")
            for dc in range(DC):
                dma(wp_s[:, dc, :], wp[dc * 128:(dc + 1) * 128, :])

            # needed only mid-attention; emitted after all startup DMA issues
            nc.gpsimd.memset(vh_s[:, :, :, dh:], 1.0)
            bp_s = cpool.tile([128, d], F32, tag="bp")
            nc.gpsimd.partition_broadcast(bp_s[:, :], bp_row[:, :])

            with tc.tile_pool(name="mm_ps", bufs=2, space="PSUM") as mmps:
                # HAM warm-up: junk matmuls keep the PE busy while the first
                # input DMAs land, so real matmuls start at the warm clock.
                wps = mmps.tile([128, 512], F32, tag="ps", name="warm")
                for w in range(14):
                    nc.tensor.matmul(
                        wps[:, :], scratch[:, 0:128], scratch[:, :],
                        start=True, stop=True,
                    )
                # ---- V projection (8 heads, N=512) ----
                for kt in range(NKT):
                    ps = mmps.tile([128, 512], F32, tag="ps", name=f"vps{kt}")
                    for ic in range(IC):
                        nc.tensor.matmul(
                            ps[:, :],
                            xv_tiles[kt][:, ic, :],
                            wv_s[:, ic, :],
                            start=(ic == 0), stop=(ic == IC - 1),
                        )
                    nc.vector.tensor_copy(
                        vh_s[:, kt, :, 0:dh],
                        ps[:, :].rearrange("p (a b) -> p a b", b=dh),
                    )
                xv_cm.__exit__(None, None, None)

                with (
                    tc.tile_pool(name="epool", bufs=4) as epool,
                    tc.tile_pool(name="sc_ps", bufs=2, space="PSUM") as scps,
                    tc.tile_pool(name="av_ps", bufs=2, space="PSUM") as avps,
                    tc.tile_pool(name="rpool", bufs=3) as rpool,
                    tc.tile_pool(name="avn", bufs=2) as avnpool,
                    tc.tile_pool(name="obuf", bufs=3) as obuf,
                ):
                    def k_proj(hp):
                        for j0, jw in kchunks:
                            ps = mmps.tile([128, 512], F32, tag="ps",
                                           name=f"kps{hp}_{j0}")
                            for ic in range(IC):
                                nc.tensor.matmul(
                                    ps[:, :jw],
                                    wk_s[:, ic, hp * 128:(hp + 1) * 128],
                                    xkT[:, ic, j0:j0 + jw],
                                    start=(ic == 0), stop=(ic == IC - 1),
                                )
                            nc.vector.tensor_copy(kh_all[:, hp, j0:j0 + jw],
                                                  ps[:, :jw])

                    def q_proj_chunk(j):
                        xT = xq_tiles[j]
                        for hp in range(HP):
                            ps = mmps.tile([128, 512], F32, tag="ps",
                                           name=f"qps{hp}_{j}")
                            for ic in range(IC):
                                nc.tensor.matmul(
                                    ps[:, :],
                                    wq_s[:, ic, hp * 128:(hp + 1) * 128],
                                    xT[:, ic, :],
                                    start=(ic == 0), stop=(ic == IC - 1),
                                )
                            nc.vector.tensor_copy(
                                qh_all[:, hp, j * QW:(j + 1) * QW], ps[:, :])

                    def emit_scores_kp(j, hp, es, kp):
                        q0 = j * QW
                        kts = [kt for kt in (2 * kp, 2 * kp + 1) if kt < NKT]
                        pss = [
                            scps.tile([128, 2, 512], F32, tag="sc",
                                      name=f"sc{hp}_{j}_{kp}_{hf}")
                            for hf in range(2)
                        ]
                        for si, kt in enumerate(kts):
                            for half, ps in enumerate(pss):
                                p0 = half * 64
                                nc.tensor.matmul(
                                    ps[:, si, :QW],
                                    kh_all[p0:p0 + 64, hp, kt * 128:(kt + 1) * 128],
                                    qh_all[p0:p0 + 64, hp, q0:q0 + QW],
                                    start=True, stop=True,
                                    tile_position=(p0, 0),
                                )
                        for half, e in enumerate(es):
                            ps = pss[half]
                            if kts[-1] < NKT - nbias and len(kts) == 2:
                                nc.scalar.activation(
                                    e[:, kts[0]:kts[0] + 2, :], ps[:, :, :QW],
                                    mybir.ActivationFunctionType.Exp,
                                    bias=0.0, scale=SCALE,
                                )
                            else:
                                for si, kt in enumerate(kts):
                                    if kt >= NKT - nbias:
                                        nc.scalar.activation(
                                            e[:, kt, :], ps[:, si, :QW],
                                            mybir.ActivationFunctionType.Exp,
                                            bias=maskb_s[:, kt:kt + 1], scale=SCALE,
                                        )
                                    else:
                                        nc.scalar.activation(
                                            e[:, kt, :], ps[:, si, :QW],
                                            mybir.ActivationFunctionType.Exp,
                                            bias=0.0, scale=SCALE,
                                        )

                    def av_half(j, hp, es, half):
                        q0 = j * QW
                        e = es[half]
                        hh = 2 * hp + half
                        av = avps.tile([128, 512], F32, tag="av",
                                       name=f"av{hp}_{j}_{half}")
                        for kt in range(NKT):
                            nc.tensor.matmul(
                                av[:, :QW],
                                vh_s[:, kt, hh, :],
                                e[:, kt, :],
                                start=(kt == 0), stop=(kt == NKT - 1),
                            )
                        d64 = rpool.tile([128, QW], F32, tag="d64",
                                         name=f"d64_{hp}_{j}_{half}")
                        nc.vector.tensor_copy(d64[64:128, :], av[64:128, :QW])
                        d0 = rpool.tile([64, QW], F32, tag="d0",
                                        name=f"d0_{hp}_{j}_{half}")
                        nc.sync.dma_start(out=d0[:, 0:256], in_=d64[64:128, 0:256])
                        nc.sync.dma_start(out=d0[:, 256:QW], in_=d64[64:128, 256:QW])
                        rbb = rpool.tile([dh, QW], F32, tag="rbb",
                                         name=f"rbb_{hp}_{j}_{half}")
                        nc.vector.reciprocal_approx_fast(out=rbb[:, :], in_=d0[:, :])
                        if half == 0:
                            nc.vector.tensor_mul(
                                avT_s[0:dh, hp, q0:q0 + QW], av[0:dh, :QW], rbb[:, :]
                            )
                        else:
                            avn = avnpool.tile([dh, QW], BF16, tag="avn",
                                               name=f"avn{hp}_{j}")
                            nc.vector.tensor_mul(avn[:, :], av[0:dh, :QW], rbb[:, :])
                            nc.sync.dma_start(
                                out=avT_s[64:128, hp, q0:q0 + 256], in_=avn[:, 0:256]
                            )
                            nc.sync.dma_start(
                                out=avT_s[64:128, hp, q0 + 256:q0 + QW],
                                in_=avn[:, 256:QW]
                            )

                    def out_proj(j):
                        nsplit = 4 if j == QC - 1 else 1
                        for qt in range(QW // 128):
                            qa = j * QW + qt * 128
                            pso = []
                            for oc, (o0, ow) in enumerate(ochunks):
                                pso.append(mmps.tile([128, 512], F32, tag="ps",
                                                     name=f"o{j}_{qt}_{o0}"))
                            for dc in range(DC):
                                for oc, (o0, ow) in enumerate(ochunks):
                                    nc.tensor.matmul(
                                        pso[oc][:, :ow],
                                        avT_s[:, dc, qa:qa + 128],
                                        wp_s[:, dc, o0:o0 + ow],
                                        start=(dc == 0), stop=(dc == DC - 1),
                                    )
                            for oc, (o0, ow) in enumerate(ochunks):
                                ot = obuf.tile([128, 512], F32, tag="ot",
                                               name=f"ot{j}_{qt}_{o0}")
                                nc.vector.tensor_add(
                                    ot[:, :ow], pso[oc][:, :ow], bp_s[:, o0:o0 + ow]
                                )
                                sw = ow // nsplit
                                for si in range(nsplit):
                                    nc.gpsimd.dma_start(
                                        out=out[qa:qa + 128,
                                                o0 + si * sw:o0 + (si + 1) * sw],
                                        in_=ot[:, si * sw:(si + 1) * sw]
                                    )

                    NKP = (NKT + 1) // 2
                    outq = []

                    def emit_segment(j, hp, prev):
                        if hp == 2 and outq:
                            out_proj(outq.pop(0))
                        es = [
                            epool.tile([128, NKT, QW], BF16, tag="e",
                                       name=f"e{hp}_{j}_{hf}")
                            for hf in range(2)
                        ]
                        fill = {min(1, NKP - 1): 0, min(3, NKP - 1): 1}
                        for kp in range(NKP):
                            emit_scores_kp(j, hp, es, kp)
                            if prev is not None and kp in fill:
                                av_half(prev[0], prev[1], prev[2], fill[kp])
                        if prev is not None and prev[1] == HP - 1:
                            outq.append(prev[0])
                        return es

                    k_proj(0)
                    q_proj_chunk(0)
                    prev = None
                    for j in range(QC):
                        for hp in range(HP):
                            es = emit_segment(j, hp, prev)
                            prev = (j, hp, es)
                            if j == 0 and hp + 1 < HP:
                                k_proj(hp + 1)
                        if j + 1 < QC:
                            q_proj_chunk(j + 1)
                            if j + 2 < QC:
                                load_xq_chunk(j + 2)
                    av_half(*prev, 0)
                    av_half(*prev, 1)
                    outq.append(prev[0])
                    for j in outq:
                        out_proj(j)

    nc.compile()
    return nc


def host_prep(q, k, v, attention_mask, Wq, Wk, Wv, Wp, bp):
    """Pack valid keys, slice + cast full inputs into per-core input maps.
    Core c: batch c//2, head-group c%2."""
    bsz, nk_full = attention_mask.shape
    idxs = [np.flatnonzero(attention_mask[b]) for b in range(bsz)]
    nv_min = min(len(ix) for ix in idxs)
    nk = max(128, -(-max(len(ix) for ix in idxs) // 128) * 128)
    nkt = nk // 128
    nbias = max(1, -(-(nk - nv_min) // 128))

    wqT = np.ascontiguousarray(Wq.T).astype(NPBF16)
    wkT = np.ascontiguousarray(Wk.T).astype(NPBF16)
    wvT = np.ascontiguousarray(Wv.T).astype(NPBF16)
    wpT = np.ascontiguousarray(Wp.T).astype(NPBF16)
    G = HG * DH

    IC = D // 128
    packed = []
    for b in range(bsz):
        ix = idxs[b]
        kp = np.zeros((nk, k.shape[2]), NPBF16)
        vp = np.zeros((nk, v.shape[2]), NPBF16)
        kp[:len(ix)] = k[b][ix].astype(NPBF16)
        vp[:len(ix)] = v[b][ix].astype(NPBF16)
        mb = np.full(nk, MASK_NEG, np.float32)
        mb[:len(ix)] = 0.0
        # kt-major xv layout: xvk[kt*128+p, ic*128+n] = v_T[ic*128+p, kt*128+n]
        xvk = np.ascontiguousarray(
            vp.T.reshape(IC, 128, nkt, 128).transpose(2, 1, 0, 3)
            .reshape(nk, D))
        packed.append((np.ascontiguousarray(kp.T), xvk,
                       np.ascontiguousarray(mb.reshape(nkt, 128).T),
                       np.ascontiguousarray(q[b].astype(NPBF16).T)))

    bp_zero = np.zeros((1, D), np.float32)
    bp_real = np.ascontiguousarray(bp[None, :]).astype(np.float32)
    in_maps = []
    for c in range(N_CORES):
        b, g = divmod(c, 2)
        kp, vp, mb, xq = packed[b]
        in_maps.append({
            "xq": xq, "xk": kp, "xv": vp,
            "wq": np.ascontiguousarray(wqT[:, g * G:(g + 1) * G]),
            "wk": np.ascontiguousarray(wkT[:, g * G:(g + 1) * G]),
            "wv": np.ascontiguousarray(wvT[:, g * G:(g + 1) * G]),
            "wp": np.ascontiguousarray(wpT[g * G:(g + 1) * G, :]),
            "maskb": mb, "bpb": bp_real if g == 0 else bp_zero,
        })
    return in_maps, nk, nbias


_NC_CACHE = {}


def get_nc(nq, nk, nbias=2):
    key = (nq, nk, nbias)
    if key not in _NC_CACHE:
        _NC_CACHE[key] = build_nc(nq, nk, nbias)
    return _NC_CACHE[key]


def unshard(results):
    out = np.empty((B, NQ_FULL, D), np.float32)
    for b in range(B):
        out[b] = results[2 * b]["out"]
        out[b] += results[2 * b + 1]["out"]
    return out


def kernel(q, k, v, attention_mask, Wq, Wk, Wv, Wp, bp):
    in_maps, nk, nbias = host_prep(q, k, v, attention_mask, Wq, Wk, Wv, Wp, bp)
    nc = get_nc(NQ_FULL, nk, nbias)
    res = run_bass_kernel_spmd(nc, in_maps, core_ids=list(range(N_CORES)))
    return unshard(res.results)
